# revision 36
# baseline (speedup 1.0000x reference)
"""Trainium2 Bass kernel for nn_CNN3_FPB (dense CNN + bypass MLP + FC head).

Data-parallel over 8 NeuronCores: batch 2048 -> 256 per core. All weights
replicated. Inside each core:

  stage0: y[p,b,c] = Wp1 @ xT[:,b,c] + Wp0 @ x0[:,b] + bp, relu
          (single K=128 matmul: [Wp1T; Wp0T] stacked against [xT; x0bcast]).
          Output h1d [128, 257, BB]: scalar writes the lower 64 partitions
          (col c = h(c-1)); SBUF->SBUF DMAs copy lower cols 1..255 to the
          upper 64 partitions cols 0..254 (col c = h(c)), one DMA per pair
          of chunks. The block loop is software-pipelined one stage
          (stage0(blk) issues before conv123(blk-1)) so the shift-DMAs hide
          under the previous block's conv matmuls.
  conv1:  K=3 stride 1: taps (0,1) merged into ONE K=128 matmul against
          h1d, tap 2 as a K=64 matmul (2 insts/chunk instead of 3).
  conv2:  K=3 stride 2, Cin=128, Cout=256 (2 M-chunks), parity-split input.
  conv3:  K=3 stride 2, Cin=256 (2 K-chunks), Cout=256 (2 M-chunks).
  fc1:    f-outer: for each of 8 f-chunks stream that chunk's bf16 weights
          (65 slabs: 64 l3 + bypass) and run 130 accumulating matmuls; the
          z2 activation and fc2 matmul for chunk f are deferred into chunk
          f+1's matmul stream so the PE never waits on them.
  fc2:    accumulated over the 8 f-chunks into one [2, BC] psum.

Tensor-engine cost is N(cols) cycles per matmul regardless of K/M, so taps
are packed to maximize K per instruction and all moving operands are
N=512 (convs) / N=256 (fc). PSUM tiles are one bank each ([128,512] conv,
[64,512] stage0, separate pools) so activation lag never blocks matmul
issue through buffer recycling. A PE warm-up spin on a memset tile flips
the HAM clock gate to 2.4GHz before the first input DMA lands.
"""

import os
import sys
from contextlib import ExitStack

import numpy as np

for _p in ("/opt/trn_rl_repo", "/root/.axon_site/_ro/trn_rl_repo"):
    if os.path.isdir(_p) and _p not in sys.path:
        sys.path.insert(0, _p)

import ml_dtypes  # noqa: E402
import concourse.bass as bass  # noqa: E402
from concourse import bacc  # noqa: E402
import concourse.mybir as mybir  # noqa: E402
import concourse.tile as tile  # noqa: E402

F32 = mybir.dt.float32
F32R = mybir.dt.float32r
BF16 = mybir.dt.bfloat16
RELU = mybir.ActivationFunctionType.Relu
ADD = mybir.AluOpType.add
MAX = mybir.AluOpType.max

# Problem constants (hardcoded; must match the grading problem).
B, CL, IL = 2048, 256, 64
NCORES = 8
BC = B // NCORES  # 256 samples per core
BB = 16           # samples per conv block
NBLK = BC // BB
PC = 64
CH1, CH2, CH3 = 128, 256, 256
L1, L2, L3 = 255, 128, 64
F1 = 1024
OUTC = 2
NSLAB = 65        # 64 l3 slabs + 1 bypass slab per f-chunk
SLABG = 13        # slabs per DMA group (5 groups)

NBIAS = 19  # bias columns: see _prep_bias

TDT = BF16


def build_nc():
    nc = bacc.Bacc()

    def memset_pad(ap):
        nc.gpsimd.memset(ap, 0.0)

    xs = nc.declare_dram_parameter("xs", [NBLK, 128, CL, BB], TDT, isOutput=False)
    x0s = nc.declare_dram_parameter("x0s", [64, BC], F32R, isOutput=False)
    # M-duplicated so stage0 matmuls share the conv's 128x128 PE geometry
    wstk = nc.declare_dram_parameter("wstk", [128, 128], TDT, isOutput=False)
    w1s = nc.declare_dram_parameter("w1s", [128, CH1], TDT, isOutput=False)
    # tap2 zero-padded to K=128: keeps conv1's two matmuls in the SAME PE
    # tile geometry (alternating 128x128/64x128 costs ~100ns reconfig each)
    w1t2 = nc.declare_dram_parameter("w1t2", [128, CH1], TDT, isOutput=False)
    w2 = nc.declare_dram_parameter("w2", [128, 3, CH2], TDT, isOutput=False)
    w3 = nc.declare_dram_parameter("w3", [128, 2, 3, CH3], TDT, isOutput=False)
    wb1 = nc.declare_dram_parameter("wb1", [64, 64], F32R, isOutput=False)
    wb2 = nc.declare_dram_parameter("wb2", [64, 128], F32R, isOutput=False)
    wb3 = nc.declare_dram_parameter("wb3", [128, 256], F32R, isOutput=False)
    # f-outer fc1 weights: [f, group, part, slab-in-group, c, fchunk]
    wgf = nc.declare_dram_parameter(
        "wgf", [8, 5, 128, SLABG, 2, 128], BF16, isOutput=False
    )
    # fc2 stationary zero-padded to M=128 bf16: stays in fc1's PE geometry
    wfc2 = nc.declare_dram_parameter("wfc2", [128, 8, 128], BF16, isOutput=False)
    bias = nc.declare_dram_parameter("bias", [128, NBIAS], F32, isOutput=False)
    out = nc.declare_dram_parameter("out", [OUTC, BC], F32, isOutput=True)

    with ExitStack() as ctx:
        tc = ctx.enter_context(tile.TileContext(nc))
        wpool = ctx.enter_context(tc.tile_pool(name="wpool", bufs=1))
        xpool = ctx.enter_context(tc.tile_pool(name="xpool", bufs=2))
        h1pool = ctx.enter_context(tc.tile_pool(name="h1pool", bufs=2))
        h2pool = ctx.enter_context(tc.tile_pool(name="h2pool", bufs=2))
        h3pool = ctx.enter_context(tc.tile_pool(name="h3pool", bufs=2))
        zpool = ctx.enter_context(tc.tile_pool(name="zpool", bufs=1))
        wgpool = ctx.enter_context(tc.tile_pool(name="wgpool", bufs=5))
        spool = ctx.enter_context(tc.tile_pool(name="spool", bufs=1))

        # ---- startup DMAs, ordered for earliest PE start ----
        spin_t = wpool.tile([128, 256], TDT)
        nc.gpsimd.memset(spin_t[:], 0.0)
        wstk_t = wpool.tile([128, 128], TDT)
        nc.sync.dma_start(wstk_t[:], wstk[:])
        # block-0 input in 4 slices aligned to stage0 pairs: pair p only
        # needs slices <= p, so compute starts after ~1/4 of the transfer
        xt_pre = {}
        t0 = xpool.tile([128, CL, BB], TDT, name="xt0", tag="xt")
        for s0_, s1_ in ((0, 66), (66, 130), (130, 194), (194, CL)):
            nc.sync.dma_start(t0[:, s0_:s1_, :], xs[0, :, s0_:s1_, :])
        xt_pre[0] = t0
        t1 = xpool.tile([128, CL, BB], TDT, name="xt1", tag="xt")
        nc.sync.dma_start(t1[:, :128, :], xs[1, :, :128, :])
        nc.sync.dma_start(t1[:, 128:, :], xs[1, :, 128:, :])
        xt_pre[1] = t1
        bias_t = wpool.tile([128, NBIAS], F32)
        nc.sync.dma_start(bias_t[:], bias[:])
        x0_t = wpool.tile([64, BC], F32R)
        nc.sync.dma_start(x0_t[:], x0s[:])
        wb1_t = wpool.tile([64, 64], F32R)
        nc.sync.dma_start(wb1_t[:], wb1[:])
        wb2_t = wpool.tile([64, 128], F32R)
        nc.sync.dma_start(wb2_t[:], wb2[:])
        wb3_t = wpool.tile([128, 256], F32R)
        nc.sync.dma_start(wb3_t[:], wb3[:])
        w1s_t = wpool.tile([128, CH1], TDT)
        nc.sync.dma_start(w1s_t[:], w1s[:])
        w1t2_t = wpool.tile([128, CH1], TDT)
        nc.sync.dma_start(w1t2_t[:], w1t2[:])
        w2_t = wpool.tile([128, 3, CH2], TDT)
        nc.sync.dma_start(w2_t[:], w2[:])
        w3_t = wpool.tile([128, 2, 3, CH3], TDT)
        nc.sync.dma_start(w3_t[:], w3[:])
        wfc2_t = wpool.tile([128, 8, 128], BF16)
        nc.sync.dma_start(wfc2_t[:], wfc2[:])

        bp_ap = bias_t[:64, 0:1]
        b1_ap = bias_t[:, 1:2]

        cpsum_ctx = ExitStack()
        cpsum = cpsum_ctx.enter_context(
            tc.tile_pool(name="cpsum", bufs=4, space="PSUM")
        )
        s0psum = cpsum_ctx.enter_context(
            tc.tile_pool(name="s0psum", bufs=4, space="PSUM")
        )

        # ---- PE warm-up spin on the memset tile: no DMA dependency, so it
        # starts right after the preamble and flips the HAM clock gate to
        # 8/8 (~2.4GHz) before the first real block.
        warm_ps = cpsum.tile([128, 512], F32, tag="ps")
        for i in range(18):
            nc.tensor.matmul(
                warm_ps[:, 256 * (i % 2) : 256 * (i % 2) + 256],
                spin_t[:, :128], spin_t[:],
                start=True, stop=True,
            )

        # ---- bypass MLP (tiny, fp32r); emitted after stage0(0) ----
        fbyp = spool.tile([128, 2, BC], BF16)

        def emit_bypass():
            ps = cpsum.tile([64, BC], F32, tag="ps")
            nc.tensor.matmul(ps[:], wb1_t[:], x0_t[:], start=True, stop=True)
            s1 = spool.tile([64, BC], F32R)
            nc.scalar.activation(s1[:], ps[:], RELU, bias=bias_t[:64, 6:7])
            ps = cpsum.tile([128, BC], F32, tag="ps")
            nc.tensor.matmul(ps[:], wb2_t[:], s1[:], start=True, stop=True)
            s2 = spool.tile([128, BC], F32R)
            nc.scalar.activation(s2[:], ps[:], RELU, bias=bias_t[:, 7:8])
            for m in range(2):
                ps = cpsum.tile([128, BC], F32, tag="ps")
                nc.tensor.matmul(
                    ps[:], wb3_t[:, m * 128 : (m + 1) * 128], s2[:],
                    start=True, stop=True,
                )
                nc.vector.tensor_scalar(
                    fbyp[:, m, :], ps[:], bias_t[:, 8 + m : 9 + m], 0.0, ADD, MAX
                )

        # ---- resident conv3 output (fc1 rhs), bf16: [ci, cich, l3, b] ----
        zres = zpool.tile([128, 2, L3, BC], BF16)

        # ---- conv trunk (chunk-granular psum: 1 bank per tile) ----
        S0_CHUNKS = [(1 + 32 * j, 32 if j < 7 else 31) for j in range(8)]
        C1_CHUNKS = [(32 * j, 32 if j < 7 else 31) for j in range(8)]

        def emit_stage0(blk, xt, h1d):
            # 8 chunk matmuls (M=128 via duplicated stationary: uniform PE
            # geometry), act per chunk reads the lower copy; shift-DMA/pair
            for c, (c0, cc) in enumerate(S0_CHUNKS):
                ps = s0psum.tile([128, 32 * BB], F32, tag="s0ps")
                nc.tensor.matmul(
                    ps[:, : cc * BB], wstk_t[:],
                    xt[:, c0 : c0 + cc, :].rearrange("p c b -> p (c b)"),
                    start=True, stop=True,
                )
                nc.scalar.activation(
                    h1d[:64, c0 : c0 + cc, :].rearrange("p c b -> p (c b)"),
                    ps[:64, : cc * BB], RELU, bias=bp_ap,
                )
                if c % 2 == 1:
                    # upper half = lower shifted one position left, per pair
                    p0 = S0_CHUNKS[c - 1][0]
                    tot = S0_CHUNKS[c - 1][1] + cc
                    nc.sync.dma_start(
                        h1d[64:128, p0 - 1 : p0 - 1 + tot, :],
                        h1d[0:64, p0 : p0 + tot, :],
                    )

        def emit_conv1(h1d, h2e, h2o):
            for c, (l0, lc) in enumerate(C1_CHUNKS):
                ps = cpsum.tile([128, 32 * BB], F32, tag="ps")
                nc.tensor.matmul(
                    ps[:, : lc * BB], w1s_t[:],
                    h1d[:, l0 : l0 + lc, :].rearrange("p l b -> p (l b)"),
                    start=True, stop=False,
                )
                nc.tensor.matmul(
                    ps[:, : lc * BB], w1t2_t[:],
                    h1d[:, l0 + 2 : l0 + 2 + lc, :]
                    .rearrange("p l b -> p (l b)"),
                    start=False, stop=True,
                )
                ps3 = ps.rearrange("p (t x) -> p t x", x=32)
                ne, no = (lc + 1) // 2, lc // 2
                nc.vector.tensor_scalar(
                    h2e[:, 16 * c : 16 * c + ne, :], ps3[:, :ne, 0:16],
                    b1_ap, 0.0, ADD, MAX,
                )
                nc.vector.tensor_scalar(
                    h2o[:, 16 * c + 1 : 16 * c + 1 + no, :], ps3[:, :no, 16:32],
                    b1_ap, 0.0, ADD, MAX,
                )

        def emit_conv2(h2e, h2o, h3e, h3o):
            for pair in range(2):
                for m in range(2):
                    for i in range(2):
                        l20 = 64 * pair + 32 * i
                        ps = cpsum.tile([128, 32 * BB], F32, tag="ps")
                        for k in range(3):
                            if k == 0:
                                rhs = h2o[:, l20 : l20 + 32, :]
                            elif k == 1:
                                rhs = h2e[:, l20 : l20 + 32, :]
                            else:
                                rhs = h2o[:, l20 + 1 : l20 + 33, :]
                            nc.tensor.matmul(
                                ps[:],
                                w2_t[:, k, m * 128 : (m + 1) * 128],
                                rhs.rearrange("p l b -> p (l b)"),
                                start=(k == 0), stop=(k == 2),
                            )
                        ps3 = ps.rearrange("p (t x) -> p t x", x=32)
                        j0 = 32 * pair + 16 * i
                        nc.scalar.activation(
                            h3e[:, m, j0 : j0 + 16, :], ps3[:, :, 0:16],
                            RELU, bias=bias_t[:, 2 + m : 3 + m],
                        )
                        nc.vector.tensor_scalar(
                            h3o[:, m, j0 + 1 : j0 + 17, :], ps3[:, :, 16:32],
                            bias_t[:, 2 + m : 3 + m], 0.0, ADD, MAX,
                        )

        def emit_conv3(blk, h3e, h3o):
            b0 = blk * BB
            for m in range(2):
                for q in range(2):
                    l30 = 32 * q
                    ps = cpsum.tile([128, 32 * BB], F32, tag="ps")
                    acc = 0
                    for c in range(2):
                        for k in range(3):
                            if k == 0:
                                rhs = h3o[:, c, l30 : l30 + 32, :]
                            elif k == 1:
                                rhs = h3e[:, c, l30 : l30 + 32, :]
                            else:
                                rhs = h3o[:, c, l30 + 1 : l30 + 33, :]
                            nc.tensor.matmul(
                                ps[:],
                                w3_t[:, c, k, m * 128 : (m + 1) * 128],
                                rhs.rearrange("p l b -> p (l b)"),
                                start=(acc == 0), stop=(acc == 5),
                            )
                            acc += 1
                    ps3 = ps.rearrange("p (l b) -> p l b", b=BB)
                    if m == 0:
                        nc.scalar.activation(
                            zres[:, m, l30 : l30 + 32, b0 : b0 + BB], ps3[:],
                            RELU, bias=bias_t[:, 4 + m : 5 + m],
                        )
                    else:
                        nc.vector.tensor_scalar(
                            zres[:, m, l30 : l30 + 32, b0 : b0 + BB], ps3[:],
                            bias_t[:, 4 + m : 5 + m], 0.0, ADD, MAX,
                        )

        _wg_pre = None
        h1d_prev = None
        for blk in range(NBLK + 1):
            if blk < NBLK:
                if blk in xt_pre:
                    xt = xt_pre[blk]
                else:
                    xt = xpool.tile([128, CL, BB], TDT, name="xt", tag="xt")
                    nc.sync.dma_start(xt[:], xs[blk, :, :, :])
                # stage0 -> h1d [128, 257, BB]:
                #   lower 64: col c = h(c-1)  (pads c=0, c=256)
                #   upper 64: col c = h(c)    (cols 0..254; via shift-DMA)
                h1d = h1pool.tile([128, L1 + 2, BB], TDT)
                memset_pad(h1d[:64, 0:1, :])
                memset_pad(h1d[:64, 256:257, :])
                # upper cols 255-256 are read (x0 of the full-K tap2 matmul)
                # but never DMA-filled; zero them so garbage can't poison it
                memset_pad(h1d[64:128, 255:257, :])
                if blk != 1:
                    emit_stage0(blk, xt, h1d)
            if blk == 1:
                emit_bypass()
            if blk > 0:
                h2e = h2pool.tile([128, 128, BB], TDT)
                h2o = h2pool.tile([128, 129, BB], TDT)
                memset_pad(h2o[:, 0:1, :])
                memset_pad(h2o[:, 128:129, :])
                emit_conv1(h1d_prev, h2e, h2o)
                h3e = h3pool.tile([128, 2, 64, BB], TDT)
                h3o = h3pool.tile([128, 2, 65, BB], TDT)
                memset_pad(h3o[:, :, 0:1, :])
                emit_conv2(h2e, h2o, h3e, h3o)
                emit_conv3(blk - 1, h3e, h3o)
            if blk == 1:
                # deferred: by now xt1's DMA has landed, so stage0(1) doesn't
                # stall the PE long enough for the HAM clock gate to drop
                emit_stage0(blk, xt, h1d)
            if blk == NBLK - 1:
                # prefetch first fc1 weight group during the last conv blocks
                _wg_pre = wgpool.tile(
                    [128, SLABG, 2, 128], BF16, name="wg_0", tag="wg"
                )
                nc.sync.dma_start(_wg_pre[:, :7], wgf[0, 0, :, :7])
                nc.sync.dma_start(_wg_pre[:, 7:], wgf[0, 0, :, 7:])
            if blk < NBLK:
                h1d_prev = h1d

        # ---- fc1 (f-outer) + deferred fc2 interleave ----
        cpsum_ctx.close()
        fpsum_ctx = ExitStack()
        fpsum = fpsum_ctx.enter_context(tc.tile_pool(name="fpsum", bufs=2, space="PSUM"))
        f2psum = fpsum_ctx.enter_context(tc.tile_pool(name="f2psum", bufs=1, space="PSUM"))
        ps2 = f2psum.tile([128, BC], F32, tag="ps2", name="ps2")
        z2 = spool.tile([128, 8, BC], BF16)

        def emit_fc2(f):
            nc.tensor.matmul(
                ps2[:], wfc2_t[:, f, :], z2[:, f, :],
                start=(f == 0), stop=(f == 7),
            )

        for f in range(8):
            fps = fpsum.tile([128, BC], F32, tag="fps")
            nmm = 0
            for g in range(5):
                if f == 0 and g == 0:
                    slab = _wg_pre
                else:
                    slab = wgpool.tile(
                        [128, SLABG, 2, 128], BF16, name=f"wg_{f * 5 + g}", tag="wg"
                    )
                    # two DMAs per group: parallel queues, finer completion
                    nc.sync.dma_start(slab[:, :7], wgf[f, g, :, :7])
                    nc.sync.dma_start(slab[:, 7:], wgf[f, g, :, 7:])
                for s in range(SLABG):
                    sg = g * SLABG + s
                    for c in range(2):
                        rhs = zres[:, c, sg, :] if sg < L3 else fbyp[:, c, :]
                        nc.tensor.matmul(
                            fps[:], slab[:, s, c, :], rhs,
                            start=(nmm == 0), stop=(nmm == 2 * NSLAB - 1),
                        )
                        nmm += 1
                        # fc2 matmul for the previous f-chunk, deferred into
                        # this chunk's stream so the z2 activation overlaps
                        if f > 0 and g == 0 and s == 4 and c == 1:
                            emit_fc2(f - 1)
            nc.scalar.activation(
                z2[:, f, :], fps[:], RELU, bias=bias_t[:, 10 + f : 11 + f]
            )
        emit_fc2(7)

        osb = spool.tile([2, BC], F32)
        nc.vector.tensor_scalar_add(osb[:], ps2[:2, :], bias_t[:2, 18:19])
        nc.sync.dma_start(out[:], osb[:])
        fpsum_ctx.close()

    nc.compile()
    return nc


def _prep_inputs(inputs):
    """Host-side layout prep. Returns per-core input maps."""
    f32 = lambda a: np.ascontiguousarray(np.asarray(a), dtype=np.float32)
    x = f32(inputs["x"])
    Wp = f32(inputs["Wp"])
    W1, W2, W3 = f32(inputs["W1"]), f32(inputs["W2"]), f32(inputs["W3"])
    Wb1, Wb2, Wb3 = f32(inputs["Wb1"]), f32(inputs["Wb2"]), f32(inputs["Wb3"])
    Wfc1, Wfc2 = f32(inputs["Wfc1"]), f32(inputs["Wfc2"])

    xr3 = x.reshape(B, CL, IL)  # [b, c, i]
    xT = np.ascontiguousarray(xr3.transpose(2, 1, 0))  # [i, c, b]
    x0T = np.ascontiguousarray(xr3[:, 0, :].T)  # [i, b]

    tnp = ml_dtypes.bfloat16
    wstk = np.concatenate([Wp[:, :, 1].T, Wp[:, :, 0].T], axis=0)  # [128, 64]
    wstk = np.concatenate([wstk, wstk], axis=1)  # M-dup -> [128, 128]
    # fc1 weights, f-outer layout [8, 5, 128, SLABG, 2, 128]
    wg = np.ascontiguousarray(
        Wfc1[:, : CH3 * L3].reshape(F1, CH3, L3).transpose(2, 1, 0)
        .reshape(L3, 2, 128, F1).transpose(0, 2, 1, 3)
    )  # [L3, 128part, 2, F1]
    wbyp = np.ascontiguousarray(
        Wfc1[:, CH3 * L3 :].T.reshape(2, 128, F1).transpose(1, 0, 2)
    )  # [128part, 2, F1]
    wall = np.concatenate([wg, wbyp[None]], axis=0)  # [65, 128, 2, F1]
    wgf = np.stack(
        [wall[:, :, :, f * 128 : (f + 1) * 128] for f in range(8)]
    ).reshape(8, 5, SLABG, 128, 2, 128).transpose(0, 1, 3, 2, 4, 5)

    shared = {
        "wstk": np.ascontiguousarray(wstk).astype(tnp),
        "w1s": np.ascontiguousarray(
            np.concatenate([W1[:, :, 0].T, W1[:, :, 1].T], axis=0)
        ).astype(tnp),
        "w1t2": np.ascontiguousarray(
            np.concatenate([W1[:, :, 2].T, np.zeros((64, CH1), np.float32)])
        ).astype(tnp),
        "w2": np.ascontiguousarray(W2.transpose(1, 2, 0)).astype(tnp),
        "w3": np.ascontiguousarray(
            W3.transpose(1, 2, 0).reshape(2, 128, 3, CH3).transpose(1, 0, 2, 3)
        ).astype(tnp),
        "wb1": np.ascontiguousarray(Wb1.T),
        "wb2": np.ascontiguousarray(Wb2.T),
        "wb3": np.ascontiguousarray(Wb3.T),
        "wgf": np.ascontiguousarray(wgf).astype(ml_dtypes.bfloat16),
        "wfc2": np.ascontiguousarray(
            np.concatenate(
                [
                    Wfc2.T.reshape(8, 128, OUTC).transpose(1, 0, 2),
                    np.zeros((128, 8, 128 - OUTC), np.float32),
                ],
                axis=2,
            )
        ).astype(tnp),
    }

    bias_np = np.zeros((128, NBIAS), np.float32)
    bias_np[:64, 0] = f32(inputs["bp"])
    bias_np[64:, 0] = f32(inputs["bp"])
    bias_np[:, 1] = f32(inputs["b1"])
    b2, b3 = f32(inputs["b2"]), f32(inputs["b3"])
    bias_np[:, 2], bias_np[:, 3] = b2[:128], b2[128:]
    bias_np[:, 4], bias_np[:, 5] = b3[:128], b3[128:]
    bias_np[:64, 6] = f32(inputs["bb1"])
    bias_np[:, 7] = f32(inputs["bb2"])
    bb3 = f32(inputs["bb3"])
    bias_np[:, 8], bias_np[:, 9] = bb3[:128], bb3[128:]
    bias_np[:, 10:18] = f32(inputs["bfc1"]).reshape(8, 128).T
    bias_np[:2, 18] = f32(inputs["bfc2"])
    shared["bias"] = bias_np

    in_maps = []
    for core in range(NCORES):
        sl = slice(core * BC, (core + 1) * BC)
        xc = xT[:, :, sl].reshape(IL, CL, NBLK, BB)
        x0b = x0T[:, sl].reshape(IL, NBLK, BB)
        xs_core = np.empty((NBLK, 128, CL, BB), tnp)
        xs_core[:, :64] = xc.transpose(2, 0, 1, 3)
        xs_core[:, 64:] = x0b.transpose(1, 0, 2)[:, :, None, :]
        m = dict(shared)
        m["xs"] = xs_core
        m["x0s"] = np.ascontiguousarray(x0T[:, sl])
        in_maps.append(m)
    return in_maps


_NC_CACHE = {}


def _get_nc():
    if "nc" not in _NC_CACHE:
        _NC_CACHE["nc"] = build_nc()
    return _NC_CACHE["nc"]


def run(inputs, trace=False):
    from concourse.bass_utils import run_bass_kernel_spmd

    nc = _get_nc()
    in_maps = _prep_inputs(inputs)
    res = run_bass_kernel_spmd(
        nc, in_maps, core_ids=list(range(NCORES)), trace=trace
    )
    outs = [np.asarray(r["out"]) for r in res.results]
    full = np.concatenate([o.T for o in outs], axis=0).astype(np.float32)
    return full, res


def kernel(**inputs) -> np.ndarray:
    full, _ = run(inputs, trace=False)
    return full


# revision 46
# speedup vs baseline: 1.1626x; 1.1626x over previous
"""Trainium2 Bass kernel for nn_CNN3_FPB (dense CNN + bypass MLP + FC head).

Data-parallel over 8 NeuronCores: batch 2048 -> 256 per core. All weights
replicated. Inside each core:

  stage0: y[p,b,c] = Wp1 @ xT[:,b,c] + Wp0 @ x0[:,b] + bp, relu
          (single K=128 matmul: [Wp1T; Wp0T] stacked against [xT; x0bcast]).
          Output h1d [128, 257, BB]: scalar writes the lower 64 partitions
          (col c = h(c-1)); SBUF->SBUF DMAs copy lower cols 1..255 to the
          upper 64 partitions cols 0..254 (col c = h(c)), one DMA per pair
          of chunks. The block loop is software-pipelined one stage
          (stage0(blk) issues before conv123(blk-1)) so the shift-DMAs hide
          under the previous block's conv matmuls.
  conv1:  K=3 stride 1: taps (0,1) merged into ONE K=128 matmul against
          h1d, tap 2 as a K=64 matmul (2 insts/chunk instead of 3).
  conv2:  K=3 stride 2, Cin=128, Cout=256 (2 M-chunks), parity-split input.
  conv3:  K=3 stride 2, Cin=256 (2 K-chunks), Cout=256 (2 M-chunks).
  fc1:    f-outer: for each of 8 f-chunks stream that chunk's bf16 weights
          (65 slabs: 64 l3 + bypass) and run 130 accumulating matmuls; the
          z2 activation and fc2 matmul for chunk f are deferred into chunk
          f+1's matmul stream so the PE never waits on them.
  fc2:    accumulated over the 8 f-chunks into one [2, BC] psum.

Tensor-engine cost is N(cols) cycles per matmul regardless of K/M, so taps
are packed to maximize K per instruction and all moving operands are
N=512 (convs) / N=256 (fc). PSUM tiles are one bank each ([128,512] conv,
[64,512] stage0, separate pools) so activation lag never blocks matmul
issue through buffer recycling. A PE warm-up spin on a memset tile flips
the HAM clock gate to 2.4GHz before the first input DMA lands.
"""

import os
import sys
from contextlib import ExitStack

import numpy as np

for _p in ("/opt/trn_rl_repo", "/root/.axon_site/_ro/trn_rl_repo"):
    if os.path.isdir(_p) and _p not in sys.path:
        sys.path.insert(0, _p)

import ml_dtypes  # noqa: E402
import concourse.bass as bass  # noqa: E402
from concourse import bacc  # noqa: E402
import concourse.mybir as mybir  # noqa: E402
import concourse.tile as tile  # noqa: E402

F32 = mybir.dt.float32
F32R = mybir.dt.float32r
BF16 = mybir.dt.bfloat16
RELU = mybir.ActivationFunctionType.Relu
ADD = mybir.AluOpType.add
MAX = mybir.AluOpType.max

# Problem constants (hardcoded; must match the grading problem).
B, CL, IL = 2048, 256, 64
NCORES = 8
BC = B // NCORES  # 256 samples per core
BB = 16           # samples per conv block
NBLK = BC // BB
PC = 64
CH1, CH2, CH3 = 128, 256, 256
L1, L2, L3 = 255, 128, 64
F1 = 1024
OUTC = 2
NSLAB = 65        # 64 l3 slabs + 1 bypass slab per f-chunk
SLABG = 13        # slabs per DMA group (5 groups)

NBIAS = 19  # bias columns: see _prep_bias

TDT = BF16


def build_nc():
    nc = bacc.Bacc()

    def memset_pad(ap):
        nc.gpsimd.memset(ap, 0.0)

    xs = nc.declare_dram_parameter("xs", [NBLK, 128, CL, BB], TDT, isOutput=False)
    x0s = nc.declare_dram_parameter("x0s", [64, BC], F32R, isOutput=False)
    wstk = nc.declare_dram_parameter("wstk", [128, 64], TDT, isOutput=False)
    w1s = nc.declare_dram_parameter("w1s", [128, CH1], TDT, isOutput=False)
    # tap2 zero-padded to K=128: keeps conv1's two matmuls in the SAME PE
    # tile geometry (alternating 128x128/64x128 costs ~100ns reconfig each)
    w1t2 = nc.declare_dram_parameter("w1t2", [128, CH1], TDT, isOutput=False)
    w2 = nc.declare_dram_parameter("w2", [128, 3, CH2], TDT, isOutput=False)
    w3 = nc.declare_dram_parameter("w3", [128, 2, 3, CH3], TDT, isOutput=False)
    wb1 = nc.declare_dram_parameter("wb1", [64, 64], F32R, isOutput=False)
    wb2 = nc.declare_dram_parameter("wb2", [64, 128], F32R, isOutput=False)
    wb3 = nc.declare_dram_parameter("wb3", [128, 256], F32R, isOutput=False)
    # f-outer fc1 weights: [f, group, part, slab-in-group, c, fchunk]
    wgf = nc.declare_dram_parameter(
        "wgf", [8, 5, 128, SLABG, 2, 128], BF16, isOutput=False
    )
    wfc2 = nc.declare_dram_parameter("wfc2", [128, 8, OUTC], F32R, isOutput=False)
    bias = nc.declare_dram_parameter("bias", [128, NBIAS], F32, isOutput=False)
    out = nc.declare_dram_parameter("out", [OUTC, BC], F32, isOutput=True)

    with ExitStack() as ctx:
        tc = ctx.enter_context(tile.TileContext(nc))
        wpool = ctx.enter_context(tc.tile_pool(name="wpool", bufs=1))
        xpool = ctx.enter_context(tc.tile_pool(name="xpool", bufs=2))
        h1pool = ctx.enter_context(tc.tile_pool(name="h1pool", bufs=2))
        h2pool = ctx.enter_context(tc.tile_pool(name="h2pool", bufs=2))
        h3pool = ctx.enter_context(tc.tile_pool(name="h3pool", bufs=2))
        zpool = ctx.enter_context(tc.tile_pool(name="zpool", bufs=1))
        wgpool = ctx.enter_context(tc.tile_pool(name="wgpool", bufs=5))
        spool = ctx.enter_context(tc.tile_pool(name="spool", bufs=1))

        # ---- startup DMAs, ordered for earliest PE start ----
        spin_t = wpool.tile([128, 256], TDT)
        nc.gpsimd.memset(spin_t[:], 0.0)
        wstk_t = wpool.tile([128, 64], TDT)
        nc.sync.dma_start(wstk_t[:], wstk[:])
        # block-0 input in 4 slices aligned to stage0 pairs: pair p only
        # needs slices <= p, so compute starts after ~1/4 of the transfer
        xt_pre = {}
        t0 = xpool.tile([128, CL, BB], TDT, name="xt0", tag="xt")
        for s0_, s1_ in ((0, 66), (66, 130), (130, 194), (194, CL)):
            nc.sync.dma_start(t0[:, s0_:s1_, :], xs[0, :, s0_:s1_, :])
        xt_pre[0] = t0
        t1 = xpool.tile([128, CL, BB], TDT, name="xt1", tag="xt")
        nc.sync.dma_start(t1[:, :128, :], xs[1, :, :128, :])
        nc.sync.dma_start(t1[:, 128:, :], xs[1, :, 128:, :])
        xt_pre[1] = t1
        bias_t = wpool.tile([128, NBIAS], F32)
        nc.sync.dma_start(bias_t[:], bias[:])
        x0_t = wpool.tile([64, BC], F32R)
        nc.sync.dma_start(x0_t[:], x0s[:])
        wb1_t = wpool.tile([64, 64], F32R)
        nc.sync.dma_start(wb1_t[:], wb1[:])
        wb2_t = wpool.tile([64, 128], F32R)
        nc.sync.dma_start(wb2_t[:], wb2[:])
        wb3_t = wpool.tile([128, 256], F32R)
        nc.sync.dma_start(wb3_t[:], wb3[:])
        w1s_t = wpool.tile([128, CH1], TDT)
        nc.sync.dma_start(w1s_t[:], w1s[:])
        w1t2_t = wpool.tile([128, CH1], TDT)
        nc.sync.dma_start(w1t2_t[:], w1t2[:])
        w2_t = wpool.tile([128, 3, CH2], TDT)
        nc.sync.dma_start(w2_t[:], w2[:])
        w3_t = wpool.tile([128, 2, 3, CH3], TDT)
        nc.sync.dma_start(w3_t[:], w3[:])
        wfc2_t = wpool.tile([128, 8, OUTC], F32R)
        nc.sync.dma_start(wfc2_t[:], wfc2[:])

        bp_ap = bias_t[:64, 0:1]
        b1_ap = bias_t[:, 1:2]

        cpsum_ctx = ExitStack()
        cpsum = cpsum_ctx.enter_context(
            tc.tile_pool(name="cpsum", bufs=4, space="PSUM")
        )
        s0psum = cpsum_ctx.enter_context(
            tc.tile_pool(name="s0psum", bufs=4, space="PSUM")
        )

        # ---- PE warm-up spin on the memset tile: no DMA dependency, so it
        # starts right after the preamble and flips the HAM clock gate to
        # 8/8 (~2.4GHz) before the first real block.
        warm_ps = cpsum.tile([128, 512], F32, tag="ps")
        for i in range(18):
            nc.tensor.matmul(
                warm_ps[:, 256 * (i % 2) : 256 * (i % 2) + 256],
                spin_t[:, :128], spin_t[:],
                start=True, stop=True,
            )

        # ---- bypass MLP (tiny, fp32r); emitted after stage0(0) ----
        fbyp = spool.tile([128, 2, BC], BF16)

        def emit_bypass():
            ps = cpsum.tile([64, BC], F32, tag="ps")
            nc.tensor.matmul(ps[:], wb1_t[:], x0_t[:], start=True, stop=True)
            s1 = spool.tile([64, BC], F32R)
            nc.scalar.activation(s1[:], ps[:], RELU, bias=bias_t[:64, 6:7])
            ps = cpsum.tile([128, BC], F32, tag="ps")
            nc.tensor.matmul(ps[:], wb2_t[:], s1[:], start=True, stop=True)
            s2 = spool.tile([128, BC], F32R)
            nc.scalar.activation(s2[:], ps[:], RELU, bias=bias_t[:, 7:8])
            for m in range(2):
                ps = cpsum.tile([128, BC], F32, tag="ps")
                nc.tensor.matmul(
                    ps[:], wb3_t[:, m * 128 : (m + 1) * 128], s2[:],
                    start=True, stop=True,
                )
                nc.vector.tensor_scalar(
                    fbyp[:, m, :], ps[:], bias_t[:, 8 + m : 9 + m], 0.0, ADD, MAX
                )

        # ---- resident conv3 output (fc1 rhs), bf16: [ci, cich, l3, b] ----
        zres = zpool.tile([128, 2, L3, BC], BF16)

        # ---- conv trunk (chunk-granular psum: 1 bank per tile) ----
        S0_CHUNKS = [(1 + 32 * j, 32 if j < 7 else 31) for j in range(8)]
        C1_CHUNKS = [(32 * j, 32 if j < 7 else 31) for j in range(8)]

        def emit_stage0(blk, xt, h1d):
            # 8 chunk matmuls, act per chunk (scalar), shift-DMA per pair
            for c, (c0, cc) in enumerate(S0_CHUNKS):
                ps = s0psum.tile([64, 32 * BB], F32, tag="s0ps")
                nc.tensor.matmul(
                    ps[:, : cc * BB], wstk_t[:],
                    xt[:, c0 : c0 + cc, :].rearrange("p c b -> p (c b)"),
                    start=True, stop=True,
                )
                nc.scalar.activation(
                    h1d[:64, c0 : c0 + cc, :].rearrange("p c b -> p (c b)"),
                    ps[:, : cc * BB], RELU, bias=bp_ap,
                )
                if c % 2 == 1:
                    # upper half = lower shifted one position left, per pair
                    p0 = S0_CHUNKS[c - 1][0]
                    tot = S0_CHUNKS[c - 1][1] + cc
                    nc.sync.dma_start(
                        h1d[64:128, p0 - 1 : p0 - 1 + tot, :],
                        h1d[0:64, p0 : p0 + tot, :],
                    )

        def emit_conv1(h1d, h2e, h2o):
            for c, (l0, lc) in enumerate(C1_CHUNKS):
                ps = cpsum.tile([128, 32 * BB], F32, tag="ps")
                nc.tensor.matmul(
                    ps[:, : lc * BB], w1s_t[:],
                    h1d[:, l0 : l0 + lc, :].rearrange("p l b -> p (l b)"),
                    start=True, stop=False,
                )
                nc.tensor.matmul(
                    ps[:, : lc * BB], w1t2_t[:],
                    h1d[:, l0 + 2 : l0 + 2 + lc, :]
                    .rearrange("p l b -> p (l b)"),
                    start=False, stop=True,
                )
                ps3 = ps.rearrange("p (t x) -> p t x", x=32)
                ne, no = (lc + 1) // 2, lc // 2
                nc.vector.tensor_scalar(
                    h2e[:, 16 * c : 16 * c + ne, :], ps3[:, :ne, 0:16],
                    b1_ap, 0.0, ADD, MAX,
                )
                nc.vector.tensor_scalar(
                    h2o[:, 16 * c + 1 : 16 * c + 1 + no, :], ps3[:, :no, 16:32],
                    b1_ap, 0.0, ADD, MAX,
                )

        def emit_conv2(h2e, h2o, h3e, h3o):
            for pair in range(2):
                for m in range(2):
                    for i in range(2):
                        l20 = 64 * pair + 32 * i
                        ps = cpsum.tile([128, 32 * BB], F32, tag="ps")
                        for k in range(3):
                            if k == 0:
                                rhs = h2o[:, l20 : l20 + 32, :]
                            elif k == 1:
                                rhs = h2e[:, l20 : l20 + 32, :]
                            else:
                                rhs = h2o[:, l20 + 1 : l20 + 33, :]
                            nc.tensor.matmul(
                                ps[:],
                                w2_t[:, k, m * 128 : (m + 1) * 128],
                                rhs.rearrange("p l b -> p (l b)"),
                                start=(k == 0), stop=(k == 2),
                            )
                        ps3 = ps.rearrange("p (t x) -> p t x", x=32)
                        j0 = 32 * pair + 16 * i
                        nc.scalar.activation(
                            h3e[:, m, j0 : j0 + 16, :], ps3[:, :, 0:16],
                            RELU, bias=bias_t[:, 2 + m : 3 + m],
                        )
                        nc.vector.tensor_scalar(
                            h3o[:, m, j0 + 1 : j0 + 17, :], ps3[:, :, 16:32],
                            bias_t[:, 2 + m : 3 + m], 0.0, ADD, MAX,
                        )

        def emit_conv3(blk, h3e, h3o):
            b0 = blk * BB
            for m in range(2):
                for q in range(2):
                    l30 = 32 * q
                    ps = cpsum.tile([128, 32 * BB], F32, tag="ps")
                    acc = 0
                    for c in range(2):
                        for k in range(3):
                            if k == 0:
                                rhs = h3o[:, c, l30 : l30 + 32, :]
                            elif k == 1:
                                rhs = h3e[:, c, l30 : l30 + 32, :]
                            else:
                                rhs = h3o[:, c, l30 + 1 : l30 + 33, :]
                            nc.tensor.matmul(
                                ps[:],
                                w3_t[:, c, k, m * 128 : (m + 1) * 128],
                                rhs.rearrange("p l b -> p (l b)"),
                                start=(acc == 0), stop=(acc == 5),
                            )
                            acc += 1
                    ps3 = ps.rearrange("p (l b) -> p l b", b=BB)
                    if m == 0:
                        nc.scalar.activation(
                            zres[:, m, l30 : l30 + 32, b0 : b0 + BB], ps3[:],
                            RELU, bias=bias_t[:, 4 + m : 5 + m],
                        )
                    else:
                        nc.vector.tensor_scalar(
                            zres[:, m, l30 : l30 + 32, b0 : b0 + BB], ps3[:],
                            bias_t[:, 4 + m : 5 + m], 0.0, ADD, MAX,
                        )

        _wg_pre = None
        h1d_prev = None
        for blk in range(NBLK + 1):
            if blk < NBLK:
                if blk in xt_pre:
                    xt = xt_pre[blk]
                else:
                    xt = xpool.tile([128, CL, BB], TDT, name="xt", tag="xt")
                    nc.sync.dma_start(xt[:], xs[blk, :, :, :])
                # stage0 -> h1d [128, 257, BB]:
                #   lower 64: col c = h(c-1)  (pads c=0, c=256)
                #   upper 64: col c = h(c)    (cols 0..254; via shift-DMA)
                h1d = h1pool.tile([128, L1 + 2, BB], TDT)
                memset_pad(h1d[:64, 0:1, :])
                memset_pad(h1d[:64, 256:257, :])
                # upper cols 255-256 are read (x0 of the full-K tap2 matmul)
                # but never DMA-filled; zero them so garbage can't poison it
                memset_pad(h1d[64:128, 255:257, :])
                emit_stage0(blk, xt, h1d)
            if blk == 1:
                emit_bypass()
            if blk > 0:
                h2e = h2pool.tile([128, 128, BB], TDT)
                h2o = h2pool.tile([128, 129, BB], TDT)
                memset_pad(h2o[:, 0:1, :])
                memset_pad(h2o[:, 128:129, :])
                emit_conv1(h1d_prev, h2e, h2o)
                h3e = h3pool.tile([128, 2, 64, BB], TDT)
                h3o = h3pool.tile([128, 2, 65, BB], TDT)
                memset_pad(h3o[:, :, 0:1, :])
                emit_conv2(h2e, h2o, h3e, h3o)
                emit_conv3(blk - 1, h3e, h3o)
            if blk == NBLK - 1:
                # prefetch first fc1 weight group during the last conv blocks
                _wg_pre = wgpool.tile(
                    [128, SLABG, 2, 128], BF16, name="wg_0", tag="wg"
                )
                nc.sync.dma_start(_wg_pre[:, :7], wgf[0, 0, :, :7])
                nc.sync.dma_start(_wg_pre[:, 7:], wgf[0, 0, :, 7:])
            if blk < NBLK:
                h1d_prev = h1d

        # ---- fc1 (f-outer) + deferred fc2 interleave ----
        cpsum_ctx.close()
        fpsum_ctx = ExitStack()
        fpsum = fpsum_ctx.enter_context(tc.tile_pool(name="fpsum", bufs=2, space="PSUM"))
        f2psum = fpsum_ctx.enter_context(tc.tile_pool(name="f2psum", bufs=1, space="PSUM"))
        ps2 = f2psum.tile([2, BC], F32, tag="ps2", name="ps2")
        z2 = spool.tile([128, 8, BC], F32R)

        def emit_fc2(f):
            nc.tensor.matmul(
                ps2[:], wfc2_t[:, f, :], z2[:, f, :],
                start=(f == 0), stop=(f == 7),
            )

        for f in range(8):
            fps = fpsum.tile([128, BC], F32, tag="fps")
            nmm = 0
            for g in range(5):
                if f == 0 and g == 0:
                    slab = _wg_pre
                else:
                    slab = wgpool.tile(
                        [128, SLABG, 2, 128], BF16, name=f"wg_{f * 5 + g}", tag="wg"
                    )
                    # two DMAs per group: parallel queues, finer completion
                    nc.sync.dma_start(slab[:, :7], wgf[f, g, :, :7])
                    nc.sync.dma_start(slab[:, 7:], wgf[f, g, :, 7:])
                for s in range(SLABG):
                    sg = g * SLABG + s
                    for c in range(2):
                        rhs = zres[:, c, sg, :] if sg < L3 else fbyp[:, c, :]
                        nc.tensor.matmul(
                            fps[:], slab[:, s, c, :], rhs,
                            start=(nmm == 0), stop=(nmm == 2 * NSLAB - 1),
                        )
                        nmm += 1
                        # fc2 matmul for the previous f-chunk, deferred into
                        # this chunk's stream so the z2 activation overlaps
                        if f > 0 and g == 0 and s == 4 and c == 1:
                            emit_fc2(f - 1)
            nc.scalar.activation(
                z2[:, f, :], fps[:], RELU, bias=bias_t[:, 10 + f : 11 + f]
            )
        emit_fc2(7)

        osb = spool.tile([2, BC], F32)
        nc.vector.tensor_scalar_add(osb[:], ps2[:], bias_t[:2, 18:19])
        nc.sync.dma_start(out[:], osb[:])
        fpsum_ctx.close()

    nc.compile()
    return nc


def _prep_inputs(inputs):
    """Host-side layout prep. Returns per-core input maps."""
    f32 = lambda a: np.ascontiguousarray(np.asarray(a), dtype=np.float32)
    x = f32(inputs["x"])
    Wp = f32(inputs["Wp"])
    W1, W2, W3 = f32(inputs["W1"]), f32(inputs["W2"]), f32(inputs["W3"])
    Wb1, Wb2, Wb3 = f32(inputs["Wb1"]), f32(inputs["Wb2"]), f32(inputs["Wb3"])
    Wfc1, Wfc2 = f32(inputs["Wfc1"]), f32(inputs["Wfc2"])

    xr3 = x.reshape(B, CL, IL)  # [b, c, i]
    xT = np.ascontiguousarray(xr3.transpose(2, 1, 0))  # [i, c, b]
    x0T = np.ascontiguousarray(xr3[:, 0, :].T)  # [i, b]

    tnp = ml_dtypes.bfloat16
    wstk = np.concatenate([Wp[:, :, 1].T, Wp[:, :, 0].T], axis=0)  # [128, 64]
    # fc1 weights, f-outer layout [8, 5, 128, SLABG, 2, 128]
    wg = np.ascontiguousarray(
        Wfc1[:, : CH3 * L3].reshape(F1, CH3, L3).transpose(2, 1, 0)
        .reshape(L3, 2, 128, F1).transpose(0, 2, 1, 3)
    )  # [L3, 128part, 2, F1]
    wbyp = np.ascontiguousarray(
        Wfc1[:, CH3 * L3 :].T.reshape(2, 128, F1).transpose(1, 0, 2)
    )  # [128part, 2, F1]
    wall = np.concatenate([wg, wbyp[None]], axis=0)  # [65, 128, 2, F1]
    wgf = np.stack(
        [wall[:, :, :, f * 128 : (f + 1) * 128] for f in range(8)]
    ).reshape(8, 5, SLABG, 128, 2, 128).transpose(0, 1, 3, 2, 4, 5)

    shared = {
        "wstk": np.ascontiguousarray(wstk).astype(tnp),
        "w1s": np.ascontiguousarray(
            np.concatenate([W1[:, :, 0].T, W1[:, :, 1].T], axis=0)
        ).astype(tnp),
        "w1t2": np.ascontiguousarray(
            np.concatenate([W1[:, :, 2].T, np.zeros((64, CH1), np.float32)])
        ).astype(tnp),
        "w2": np.ascontiguousarray(W2.transpose(1, 2, 0)).astype(tnp),
        "w3": np.ascontiguousarray(
            W3.transpose(1, 2, 0).reshape(2, 128, 3, CH3).transpose(1, 0, 2, 3)
        ).astype(tnp),
        "wb1": np.ascontiguousarray(Wb1.T),
        "wb2": np.ascontiguousarray(Wb2.T),
        "wb3": np.ascontiguousarray(Wb3.T),
        "wgf": np.ascontiguousarray(wgf).astype(ml_dtypes.bfloat16),
        "wfc2": np.ascontiguousarray(
            Wfc2.T.reshape(8, 128, OUTC).transpose(1, 0, 2)
        ),
    }

    bias_np = np.zeros((128, NBIAS), np.float32)
    bias_np[:64, 0] = f32(inputs["bp"])
    bias_np[64:, 0] = f32(inputs["bp"])
    bias_np[:, 1] = f32(inputs["b1"])
    b2, b3 = f32(inputs["b2"]), f32(inputs["b3"])
    bias_np[:, 2], bias_np[:, 3] = b2[:128], b2[128:]
    bias_np[:, 4], bias_np[:, 5] = b3[:128], b3[128:]
    bias_np[:64, 6] = f32(inputs["bb1"])
    bias_np[:, 7] = f32(inputs["bb2"])
    bb3 = f32(inputs["bb3"])
    bias_np[:, 8], bias_np[:, 9] = bb3[:128], bb3[128:]
    bias_np[:, 10:18] = f32(inputs["bfc1"]).reshape(8, 128).T
    bias_np[:2, 18] = f32(inputs["bfc2"])
    shared["bias"] = bias_np

    in_maps = []
    for core in range(NCORES):
        sl = slice(core * BC, (core + 1) * BC)
        xc = xT[:, :, sl].reshape(IL, CL, NBLK, BB)
        x0b = x0T[:, sl].reshape(IL, NBLK, BB)
        xs_core = np.empty((NBLK, 128, CL, BB), tnp)
        xs_core[:, :64] = xc.transpose(2, 0, 1, 3)
        xs_core[:, 64:] = x0b.transpose(1, 0, 2)[:, :, None, :]
        m = dict(shared)
        m["xs"] = xs_core
        m["x0s"] = np.ascontiguousarray(x0T[:, sl])
        in_maps.append(m)
    return in_maps


_NC_CACHE = {}


def _get_nc():
    if "nc" not in _NC_CACHE:
        _NC_CACHE["nc"] = build_nc()
    return _NC_CACHE["nc"]


def run(inputs, trace=False):
    from concourse.bass_utils import run_bass_kernel_spmd

    nc = _get_nc()
    in_maps = _prep_inputs(inputs)
    res = run_bass_kernel_spmd(
        nc, in_maps, core_ids=list(range(NCORES)), trace=trace
    )
    outs = [np.asarray(r["out"]) for r in res.results]
    full = np.concatenate([o.T for o in outs], axis=0).astype(np.float32)
    return full, res


def kernel(**inputs) -> np.ndarray:
    full, _ = run(inputs, trace=False)
    return full


# revision 57
# speedup vs baseline: 1.1757x; 1.0113x over previous
"""Trainium2 Bass kernel for nn_CNN3_FPB (dense CNN + bypass MLP + FC head).

Data-parallel over 8 NeuronCores: batch 2048 -> 256 per core. All weights
replicated. Inside each core:

  stage0: y[p,b,c] = Wp1 @ xT[:,b,c] + Wp0 @ x0[:,b] + bp, relu
          (single K=128 matmul: [Wp1T; Wp0T] stacked against [xT; x0bcast]).
          Output h1d [128, 257, BB]: scalar writes the lower 64 partitions
          (col c = h(c-1)); SBUF->SBUF DMAs copy lower cols 1..255 to the
          upper 64 partitions cols 0..254 (col c = h(c)), one DMA per pair
          of chunks. The block loop is software-pipelined one stage
          (stage0(blk) issues before conv123(blk-1)) so the shift-DMAs hide
          under the previous block's conv matmuls.
  conv1:  K=3 stride 1: taps (0,1) merged into ONE K=128 matmul against
          h1d, tap 2 as a K=64 matmul (2 insts/chunk instead of 3).
  conv2:  K=3 stride 2, Cin=128, Cout=256 (2 M-chunks), parity-split input.
  conv3:  K=3 stride 2, Cin=256 (2 K-chunks), Cout=256 (2 M-chunks).
  fc1:    f-outer: for each of 8 f-chunks stream that chunk's bf16 weights
          (65 slabs: 64 l3 + bypass) and run 130 accumulating matmuls; the
          z2 activation and fc2 matmul for chunk f are deferred into chunk
          f+1's matmul stream so the PE never waits on them.
  fc2:    accumulated over the 8 f-chunks into one [2, BC] psum.

Tensor-engine cost is N(cols) cycles per matmul regardless of K/M, so taps
are packed to maximize K per instruction and all moving operands are
N=512 (convs) / N=256 (fc). PSUM tiles are one bank each ([128,512] conv,
[64,512] stage0, separate pools) so activation lag never blocks matmul
issue through buffer recycling. A PE warm-up spin on a memset tile flips
the HAM clock gate to 2.4GHz before the first input DMA lands.
"""

import os
import sys
from contextlib import ExitStack

import numpy as np

for _p in ("/opt/trn_rl_repo", "/root/.axon_site/_ro/trn_rl_repo"):
    if os.path.isdir(_p) and _p not in sys.path:
        sys.path.insert(0, _p)

import ml_dtypes  # noqa: E402
import concourse.bass as bass  # noqa: E402
from concourse import bacc  # noqa: E402
import concourse.mybir as mybir  # noqa: E402
import concourse.tile as tile  # noqa: E402

F32 = mybir.dt.float32
F32R = mybir.dt.float32r
BF16 = mybir.dt.bfloat16
RELU = mybir.ActivationFunctionType.Relu
ADD = mybir.AluOpType.add
MAX = mybir.AluOpType.max

# Problem constants (hardcoded; must match the grading problem).
B, CL, IL = 2048, 256, 64
NCORES = 8
BC = B // NCORES  # 256 samples per core
BB = 16           # samples per conv block
NBLK = BC // BB
PC = 64
CH1, CH2, CH3 = 128, 256, 256
L1, L2, L3 = 255, 128, 64
F1 = 1024
OUTC = 2
NSLAB = 65        # 64 l3 slabs + 1 bypass slab per f-chunk
SLABG = 13        # slabs per DMA group (5 groups)

NBIAS = 19  # bias columns: see _prep_bias

TDT = BF16


def build_nc():
    nc = bacc.Bacc()

    def memset_pad(ap):
        nc.gpsimd.memset(ap, 0.0)

    xs = nc.declare_dram_parameter("xs", [NBLK, 128, CL, BB], TDT, isOutput=False)
    x0s = nc.declare_dram_parameter("x0s", [64, BC], F32R, isOutput=False)
    # M-duplicated so stage0 matmuls share the conv's 128x128 PE geometry
    wstk = nc.declare_dram_parameter("wstk", [128, 128], TDT, isOutput=False)
    w1s = nc.declare_dram_parameter("w1s", [128, CH1], TDT, isOutput=False)
    # tap2 zero-padded to K=128: keeps conv1's two matmuls in the SAME PE
    # tile geometry (alternating 128x128/64x128 costs ~100ns reconfig each)
    w1t2 = nc.declare_dram_parameter("w1t2", [128, CH1], TDT, isOutput=False)
    w2 = nc.declare_dram_parameter("w2", [128, 3, CH2], TDT, isOutput=False)
    w3 = nc.declare_dram_parameter("w3", [128, 2, 3, CH3], TDT, isOutput=False)
    wb1 = nc.declare_dram_parameter("wb1", [64, 64], F32R, isOutput=False)
    wb2 = nc.declare_dram_parameter("wb2", [64, 128], F32R, isOutput=False)
    wb3 = nc.declare_dram_parameter("wb3", [128, 256], F32R, isOutput=False)
    # f-outer fc1 weights: [f, group, part, slab-in-group, c, fchunk]
    wgf = nc.declare_dram_parameter(
        "wgf", [8, 5, 128, SLABG, 2, 128], BF16, isOutput=False
    )
    # fc2 stationary zero-padded to M=128 bf16: stays in fc1's PE geometry
    wfc2 = nc.declare_dram_parameter("wfc2", [128, 8, 128], BF16, isOutput=False)
    bias = nc.declare_dram_parameter("bias", [128, NBIAS], F32, isOutput=False)
    out = nc.declare_dram_parameter("out", [OUTC, BC], F32, isOutput=True)

    with ExitStack() as ctx:
        tc = ctx.enter_context(tile.TileContext(nc))
        wpool = ctx.enter_context(tc.tile_pool(name="wpool", bufs=1))
        xpool = ctx.enter_context(tc.tile_pool(name="xpool", bufs=3))
        h1pool = ctx.enter_context(tc.tile_pool(name="h1pool", bufs=2))
        h2pool = ctx.enter_context(tc.tile_pool(name="h2pool", bufs=2))
        h3pool = ctx.enter_context(tc.tile_pool(name="h3pool", bufs=2))
        zpool = ctx.enter_context(tc.tile_pool(name="zpool", bufs=1))
        wgpool = ctx.enter_context(tc.tile_pool(name="wgpool", bufs=5))
        spool = ctx.enter_context(tc.tile_pool(name="spool", bufs=1))

        # ---- startup DMAs, ordered for earliest PE start ----
        spin_t = wpool.tile([128, 256], TDT)
        nc.gpsimd.memset(spin_t[:], 0.0)
        wstk_t = wpool.tile([128, 128], TDT)
        nc.sync.dma_start(wstk_t[:], wstk[:])
        # block-0 input in 4 slices aligned to stage0 pairs: pair p only
        # needs slices <= p, so compute starts after ~1/4 of the transfer
        xt_pre = {}
        t0 = xpool.tile([128, CL, BB], TDT, name="xt0", tag="xt")
        for s0_, s1_ in ((0, 66), (66, 130), (130, 194), (194, CL)):
            nc.sync.dma_start(t0[:, s0_:s1_, :], xs[0, :, s0_:s1_, :])
        xt_pre[0] = t0
        t1 = xpool.tile([128, CL, BB], TDT, name="xt1", tag="xt")
        for s0_, s1_ in ((0, 66), (66, 130), (130, 194), (194, CL)):
            nc.sync.dma_start(t1[:, s0_:s1_, :], xs[1, :, s0_:s1_, :])
        xt_pre[1] = t1
        bias_t = wpool.tile([128, NBIAS], F32)
        nc.sync.dma_start(bias_t[:], bias[:])
        x0_t = wpool.tile([64, BC], F32R)
        nc.sync.dma_start(x0_t[:], x0s[:])
        wb1_t = wpool.tile([64, 64], F32R)
        nc.sync.dma_start(wb1_t[:], wb1[:])
        wb2_t = wpool.tile([64, 128], F32R)
        nc.sync.dma_start(wb2_t[:], wb2[:])
        wb3_t = wpool.tile([128, 256], F32R)
        nc.sync.dma_start(wb3_t[:], wb3[:])
        w1s_t = wpool.tile([128, CH1], TDT)
        nc.sync.dma_start(w1s_t[:], w1s[:])
        w1t2_t = wpool.tile([128, CH1], TDT)
        nc.sync.dma_start(w1t2_t[:], w1t2[:])
        w2_t = wpool.tile([128, 3, CH2], TDT)
        nc.sync.dma_start(w2_t[:], w2[:])
        w3_t = wpool.tile([128, 2, 3, CH3], TDT)
        nc.sync.dma_start(w3_t[:], w3[:])
        wfc2_t = wpool.tile([128, 8, 128], BF16)
        nc.sync.dma_start(wfc2_t[:], wfc2[:])

        bp_ap = bias_t[:64, 0:1]
        b1_ap = bias_t[:, 1:2]

        cpsum_ctx = ExitStack()
        cpsum = cpsum_ctx.enter_context(
            tc.tile_pool(name="cpsum", bufs=4, space="PSUM")
        )
        s0psum = cpsum_ctx.enter_context(
            tc.tile_pool(name="s0psum", bufs=4, space="PSUM")
        )

        # ---- PE warm-up spin on the memset tile: no DMA dependency, so it
        # starts right after the preamble and flips the HAM clock gate to
        # 8/8 (~2.4GHz) before the first real block.
        warm_ps = cpsum.tile([128, 512], F32, tag="ps")
        for i in range(18):
            nc.tensor.matmul(
                warm_ps[:, 256 * (i % 2) : 256 * (i % 2) + 256],
                spin_t[:, :128], spin_t[:],
                start=True, stop=True,
            )

        # ---- bypass MLP (tiny, fp32r); emitted after stage0(0) ----
        fbyp = spool.tile([128, 2, BC], BF16)

        def emit_bypass():
            ps = cpsum.tile([64, BC], F32, tag="ps")
            nc.tensor.matmul(ps[:], wb1_t[:], x0_t[:], start=True, stop=True)
            s1 = spool.tile([64, BC], F32R)
            nc.scalar.activation(s1[:], ps[:], RELU, bias=bias_t[:64, 6:7])
            ps = cpsum.tile([128, BC], F32, tag="ps")
            nc.tensor.matmul(ps[:], wb2_t[:], s1[:], start=True, stop=True)
            s2 = spool.tile([128, BC], F32R)
            nc.scalar.activation(s2[:], ps[:], RELU, bias=bias_t[:, 7:8])
            for m in range(2):
                ps = cpsum.tile([128, BC], F32, tag="ps")
                nc.tensor.matmul(
                    ps[:], wb3_t[:, m * 128 : (m + 1) * 128], s2[:],
                    start=True, stop=True,
                )
                nc.vector.tensor_scalar(
                    fbyp[:, m, :], ps[:], bias_t[:, 8 + m : 9 + m], 0.0, ADD, MAX
                )

        # ---- resident conv3 output (fc1 rhs), bf16: [ci, cich, l3, b] ----
        zres = zpool.tile([128, 2, L3, BC], BF16)

        # ---- conv trunk (chunk-granular psum: 1 bank per tile) ----
        S0_CHUNKS = [(1 + 32 * j, 32 if j < 7 else 31) for j in range(8)]
        C1_CHUNKS = [(32 * j, 32 if j < 7 else 31) for j in range(8)]

        def emit_stage0(blk, xt, h1d):
            # 8 chunk matmuls, act per chunk (scalar), shift-DMA per pair
            for c, (c0, cc) in enumerate(S0_CHUNKS):
                ps = s0psum.tile([128, 32 * BB], F32, tag="s0ps")
                nc.tensor.matmul(
                    ps[:, : cc * BB], wstk_t[:],
                    xt[:, c0 : c0 + cc, :].rearrange("p c b -> p (c b)"),
                    start=True, stop=True,
                )
                nc.scalar.activation(
                    h1d[:64, c0 : c0 + cc, :].rearrange("p c b -> p (c b)"),
                    ps[:64, : cc * BB], RELU, bias=bp_ap,
                )
                if c % 2 == 1:
                    # upper half = lower shifted one position left, per pair
                    p0 = S0_CHUNKS[c - 1][0]
                    tot = S0_CHUNKS[c - 1][1] + cc
                    nc.sync.dma_start(
                        h1d[64:128, p0 - 1 : p0 - 1 + tot, :],
                        h1d[0:64, p0 : p0 + tot, :],
                    )

        def emit_conv1(h1d, h2e, h2o):
            for c, (l0, lc) in enumerate(C1_CHUNKS):
                ps = cpsum.tile([128, 32 * BB], F32, tag="ps")
                nc.tensor.matmul(
                    ps[:, : lc * BB], w1s_t[:],
                    h1d[:, l0 : l0 + lc, :].rearrange("p l b -> p (l b)"),
                    start=True, stop=False,
                )
                nc.tensor.matmul(
                    ps[:, : lc * BB], w1t2_t[:],
                    h1d[:, l0 + 2 : l0 + 2 + lc, :]
                    .rearrange("p l b -> p (l b)"),
                    start=False, stop=True,
                )
                ps3 = ps.rearrange("p (t x) -> p t x", x=32)
                ne, no = (lc + 1) // 2, lc // 2
                nc.vector.tensor_scalar(
                    h2e[:, 16 * c : 16 * c + ne, :], ps3[:, :ne, 0:16],
                    b1_ap, 0.0, ADD, MAX,
                )
                nc.vector.tensor_scalar(
                    h2o[:, 16 * c + 1 : 16 * c + 1 + no, :], ps3[:, :no, 16:32],
                    b1_ap, 0.0, ADD, MAX,
                )

        def emit_conv2(h2e, h2o, h3e, h3o):
            for pair in range(2):
                for m in range(2):
                    for i in range(2):
                        l20 = 64 * pair + 32 * i
                        ps = cpsum.tile([128, 32 * BB], F32, tag="ps")
                        for k in range(3):
                            if k == 0:
                                rhs = h2o[:, l20 : l20 + 32, :]
                            elif k == 1:
                                rhs = h2e[:, l20 : l20 + 32, :]
                            else:
                                rhs = h2o[:, l20 + 1 : l20 + 33, :]
                            nc.tensor.matmul(
                                ps[:],
                                w2_t[:, k, m * 128 : (m + 1) * 128],
                                rhs.rearrange("p l b -> p (l b)"),
                                start=(k == 0), stop=(k == 2),
                            )
                        ps3 = ps.rearrange("p (t x) -> p t x", x=32)
                        j0 = 32 * pair + 16 * i
                        nc.scalar.activation(
                            h3e[:, m, j0 : j0 + 16, :], ps3[:, :, 0:16],
                            RELU, bias=bias_t[:, 2 + m : 3 + m],
                        )
                        nc.vector.tensor_scalar(
                            h3o[:, m, j0 + 1 : j0 + 17, :], ps3[:, :, 16:32],
                            bias_t[:, 2 + m : 3 + m], 0.0, ADD, MAX,
                        )

        def emit_conv3(blk, h3e, h3o):
            b0 = blk * BB
            for m in range(2):
                for q in range(2):
                    l30 = 32 * q
                    ps = cpsum.tile([128, 32 * BB], F32, tag="ps")
                    acc = 0
                    for c in range(2):
                        for k in range(3):
                            if k == 0:
                                rhs = h3o[:, c, l30 : l30 + 32, :]
                            elif k == 1:
                                rhs = h3e[:, c, l30 : l30 + 32, :]
                            else:
                                rhs = h3o[:, c, l30 + 1 : l30 + 33, :]
                            nc.tensor.matmul(
                                ps[:],
                                w3_t[:, c, k, m * 128 : (m + 1) * 128],
                                rhs.rearrange("p l b -> p (l b)"),
                                start=(acc == 0), stop=(acc == 5),
                            )
                            acc += 1
                    ps3 = ps.rearrange("p (l b) -> p l b", b=BB)
                    if m == 0:
                        nc.scalar.activation(
                            zres[:, m, l30 : l30 + 32, b0 : b0 + BB], ps3[:],
                            RELU, bias=bias_t[:, 4 + m : 5 + m],
                        )
                    else:
                        nc.vector.tensor_scalar(
                            zres[:, m, l30 : l30 + 32, b0 : b0 + BB], ps3[:],
                            bias_t[:, 4 + m : 5 + m], 0.0, ADD, MAX,
                        )

        _wg_pre = None
        h1d_prev = None
        for blk in range(NBLK + 1):
            if blk < NBLK:
                if blk in xt_pre:
                    xt = xt_pre[blk]
                else:
                    xt = xpool.tile([128, CL, BB], TDT, name="xt", tag="xt")
                    nc.sync.dma_start(xt[:], xs[blk, :, :, :])
                # stage0 -> h1d [128, 257, BB]:
                #   lower 64: col c = h(c-1)  (pads c=0, c=256)
                #   upper 64: col c = h(c)    (cols 0..254; via shift-DMA)
                h1d = h1pool.tile([128, L1 + 2, BB], TDT)
                memset_pad(h1d[:64, 0:1, :])
                memset_pad(h1d[:64, 256:257, :])
                # upper cols 255-256 are read (x0 of the full-K tap2 matmul)
                # but never DMA-filled; zero them so garbage can't poison it
                memset_pad(h1d[64:128, 255:257, :])
                emit_stage0(blk, xt, h1d)
            if blk == 1:
                emit_bypass()
            if blk > 0:
                h2e = h2pool.tile([128, 128, BB], TDT)
                h2o = h2pool.tile([128, 129, BB], TDT)
                memset_pad(h2o[:, 0:1, :])
                memset_pad(h2o[:, 128:129, :])
                emit_conv1(h1d_prev, h2e, h2o)
                h3e = h3pool.tile([128, 2, 64, BB], TDT)
                h3o = h3pool.tile([128, 2, 65, BB], TDT)
                memset_pad(h3o[:, :, 0:1, :])
                emit_conv2(h2e, h2o, h3e, h3o)
                emit_conv3(blk - 1, h3e, h3o)
            if blk == NBLK - 1:
                # prefetch first fc1 weight group during the last conv blocks
                _wg_pre = wgpool.tile(
                    [128, SLABG, 2, 128], BF16, name="wg_0", tag="wg"
                )
                nc.sync.dma_start(_wg_pre[:, :7], wgf[0, 0, :, :7])
                nc.sync.dma_start(_wg_pre[:, 7:], wgf[0, 0, :, 7:])
            if blk < NBLK:
                h1d_prev = h1d

        # ---- fc1 (f-outer) + deferred fc2 interleave ----
        cpsum_ctx.close()
        fpsum_ctx = ExitStack()
        fpsum = fpsum_ctx.enter_context(tc.tile_pool(name="fpsum", bufs=2, space="PSUM"))
        f2psum = fpsum_ctx.enter_context(tc.tile_pool(name="f2psum", bufs=1, space="PSUM"))
        ps2 = f2psum.tile([128, BC], F32, tag="ps2", name="ps2")
        z2 = spool.tile([128, 8, BC], BF16)

        def emit_fc2(f):
            nc.tensor.matmul(
                ps2[:], wfc2_t[:, f, :], z2[:, f, :],
                start=(f == 0), stop=(f == 7),
            )

        for f in range(8):
            fps = fpsum.tile([128, BC], F32, tag="fps")
            nmm = 0
            for g in range(5):
                if f == 0 and g == 0:
                    slab = _wg_pre
                else:
                    slab = wgpool.tile(
                        [128, SLABG, 2, 128], BF16, name=f"wg_{f * 5 + g}", tag="wg"
                    )
                    # two DMAs per group: parallel queues, finer completion
                    nc.sync.dma_start(slab[:, :7], wgf[f, g, :, :7])
                    nc.sync.dma_start(slab[:, 7:], wgf[f, g, :, 7:])
                for s in range(SLABG):
                    sg = g * SLABG + s
                    for c in range(2):
                        rhs = zres[:, c, sg, :] if sg < L3 else fbyp[:, c, :]
                        nc.tensor.matmul(
                            fps[:], slab[:, s, c, :], rhs,
                            start=(nmm == 0), stop=(nmm == 2 * NSLAB - 1),
                        )
                        nmm += 1
                        # fc2 matmul for the previous f-chunk, deferred into
                        # this chunk's stream so the z2 activation overlaps
                        if f > 0 and g == 0 and s == 4 and c == 1:
                            emit_fc2(f - 1)
            nc.scalar.activation(
                z2[:, f, :], fps[:], RELU, bias=bias_t[:, 10 + f : 11 + f]
            )
        emit_fc2(7)

        osb = spool.tile([2, BC], F32)
        nc.vector.tensor_scalar_add(osb[:], ps2[:2, :], bias_t[:2, 18:19])
        nc.sync.dma_start(out[:], osb[:])
        fpsum_ctx.close()

    nc.compile()
    return nc


def _prep_inputs(inputs):
    """Host-side layout prep. Returns per-core input maps."""
    f32 = lambda a: np.ascontiguousarray(np.asarray(a), dtype=np.float32)
    x = f32(inputs["x"])
    Wp = f32(inputs["Wp"])
    W1, W2, W3 = f32(inputs["W1"]), f32(inputs["W2"]), f32(inputs["W3"])
    Wb1, Wb2, Wb3 = f32(inputs["Wb1"]), f32(inputs["Wb2"]), f32(inputs["Wb3"])
    Wfc1, Wfc2 = f32(inputs["Wfc1"]), f32(inputs["Wfc2"])

    xr3 = x.reshape(B, CL, IL)  # [b, c, i]
    xT = np.ascontiguousarray(xr3.transpose(2, 1, 0))  # [i, c, b]
    x0T = np.ascontiguousarray(xr3[:, 0, :].T)  # [i, b]

    tnp = ml_dtypes.bfloat16
    wstk = np.concatenate([Wp[:, :, 1].T, Wp[:, :, 0].T], axis=0)  # [128, 64]
    wstk = np.concatenate([wstk, wstk], axis=1)  # M-dup -> [128, 128]
    # fc1 weights, f-outer layout [8, 5, 128, SLABG, 2, 128]
    wg = np.ascontiguousarray(
        Wfc1[:, : CH3 * L3].reshape(F1, CH3, L3).transpose(2, 1, 0)
        .reshape(L3, 2, 128, F1).transpose(0, 2, 1, 3)
    )  # [L3, 128part, 2, F1]
    wbyp = np.ascontiguousarray(
        Wfc1[:, CH3 * L3 :].T.reshape(2, 128, F1).transpose(1, 0, 2)
    )  # [128part, 2, F1]
    wall = np.concatenate([wg, wbyp[None]], axis=0)  # [65, 128, 2, F1]
    wgf = np.stack(
        [wall[:, :, :, f * 128 : (f + 1) * 128] for f in range(8)]
    ).reshape(8, 5, SLABG, 128, 2, 128).transpose(0, 1, 3, 2, 4, 5)

    shared = {
        "wstk": np.ascontiguousarray(wstk).astype(tnp),
        "w1s": np.ascontiguousarray(
            np.concatenate([W1[:, :, 0].T, W1[:, :, 1].T], axis=0)
        ).astype(tnp),
        "w1t2": np.ascontiguousarray(
            np.concatenate([W1[:, :, 2].T, np.zeros((64, CH1), np.float32)])
        ).astype(tnp),
        "w2": np.ascontiguousarray(W2.transpose(1, 2, 0)).astype(tnp),
        "w3": np.ascontiguousarray(
            W3.transpose(1, 2, 0).reshape(2, 128, 3, CH3).transpose(1, 0, 2, 3)
        ).astype(tnp),
        "wb1": np.ascontiguousarray(Wb1.T),
        "wb2": np.ascontiguousarray(Wb2.T),
        "wb3": np.ascontiguousarray(Wb3.T),
        "wgf": np.ascontiguousarray(wgf).astype(ml_dtypes.bfloat16),
        "wfc2": np.ascontiguousarray(
            np.concatenate(
                [
                    Wfc2.T.reshape(8, 128, OUTC).transpose(1, 0, 2),
                    np.zeros((128, 8, 128 - OUTC), np.float32),
                ],
                axis=2,
            )
        ).astype(tnp),
    }

    bias_np = np.zeros((128, NBIAS), np.float32)
    bias_np[:64, 0] = f32(inputs["bp"])
    bias_np[64:, 0] = f32(inputs["bp"])
    bias_np[:, 1] = f32(inputs["b1"])
    b2, b3 = f32(inputs["b2"]), f32(inputs["b3"])
    bias_np[:, 2], bias_np[:, 3] = b2[:128], b2[128:]
    bias_np[:, 4], bias_np[:, 5] = b3[:128], b3[128:]
    bias_np[:64, 6] = f32(inputs["bb1"])
    bias_np[:, 7] = f32(inputs["bb2"])
    bb3 = f32(inputs["bb3"])
    bias_np[:, 8], bias_np[:, 9] = bb3[:128], bb3[128:]
    bias_np[:, 10:18] = f32(inputs["bfc1"]).reshape(8, 128).T
    bias_np[:2, 18] = f32(inputs["bfc2"])
    shared["bias"] = bias_np

    in_maps = []
    for core in range(NCORES):
        sl = slice(core * BC, (core + 1) * BC)
        xc = xT[:, :, sl].reshape(IL, CL, NBLK, BB)
        x0b = x0T[:, sl].reshape(IL, NBLK, BB)
        xs_core = np.empty((NBLK, 128, CL, BB), tnp)
        xs_core[:, :64] = xc.transpose(2, 0, 1, 3)
        xs_core[:, 64:] = x0b.transpose(1, 0, 2)[:, :, None, :]
        m = dict(shared)
        m["xs"] = xs_core
        m["x0s"] = np.ascontiguousarray(x0T[:, sl])
        in_maps.append(m)
    return in_maps


_NC_CACHE = {}


def _get_nc():
    if "nc" not in _NC_CACHE:
        _NC_CACHE["nc"] = build_nc()
    return _NC_CACHE["nc"]


def run(inputs, trace=False):
    from concourse.bass_utils import run_bass_kernel_spmd

    nc = _get_nc()
    in_maps = _prep_inputs(inputs)
    res = run_bass_kernel_spmd(
        nc, in_maps, core_ids=list(range(NCORES)), trace=trace
    )
    outs = [np.asarray(r["out"]) for r in res.results]
    full = np.concatenate([o.T for o in outs], axis=0).astype(np.float32)
    return full, res


def kernel(**inputs) -> np.ndarray:
    full, _ = run(inputs, trace=False)
    return full
